# revision 1
# baseline (speedup 1.0000x reference)
"""Trainium2 Bass kernel for nn_NeighborhoodSearch (sparse_attention).

Sharding: 8 cores = (batch b in {0,1}) x (head-pair hp in {0..3}); each core
computes a full-[N, D] partial contribution of its 2 heads through its slice
of Wo; the host sums the 4 partials per batch (and transposes back).

Math notes (validated against the reference in fp64):
 - The neighborhood "attention" softmax is over a singleton axis -> weights
   are all 1, so fused = sum of the 9 padded neighbors of x2 on the 48x48
   grid.  Padding slots replicate the first valid neighbor, which is
   (max(r-1,0), max(c-1,0)), with multiplicity 9 - vh(r)*vw(c).  This makes
   fused = boxsum3x3_zeropad(x2) + w2(r,c) * x2[max(r-1,0), max(c-1,0)]
   with w2 = 3*eh + 3*ew - eh*ew  (eh/ew = 1 at grid edges) in {0, 3, 5}.
 - LayerNorm feeds only the q projection, so it is folded into it:
   q = rstd[n] * (F @ (gamma*WqT*s)) - (rstd*mu)[n] * sum_d(gamma*WqT*s)
       + (beta @ WqT*s + bq*s)
 - Softmax needs no max-subtraction (scores are ~N(0, 0.3), |s| < ~3).
   The denominator comes free from an appended ones-column on v.

Device layout is feature-transposed ([D, N]); the host pre-transposes
x1/x2 and post-transposes the output.
"""

import sys

sys.path.insert(0, "/opt/trn_rl_repo")

import numpy as np

import concourse.bass as bass
import concourse.mybir as mybir
import concourse.tile as tile
from concourse.bass_utils import run_bass_kernel_spmd
from concourse.masks import make_identity

# ---------------------------------------------------------------- constants
B = 2
N = 2304          # sequence length = 48*48
D = 768           # model dim
G = 48            # grid side
P = 128           # partitions
DC = D // P       # 6 feature chunks
HD = 96           # head dim
HPC = 2           # heads per core
NQB = 384         # n-block width (matmul moving free dim)
NB = N // NQB     # 6 n-blocks
NKC = N // P      # 18 key chunks
EPS = 1e-5
QSCALE = HD ** -0.5

F32 = mybir.dt.float32
F32R = mybir.dt.float32r  # fast fp32 matmul mode (1 cyc/row at free>=256);
# operands of f32r matmuls must be produced (rounded) as f32r, so every
# tensor feeding a matmul is declared f32r end-to-end (bit-identical to f32
# in DRAM / numpy).

ADD = mybir.AluOpType.add
SUB = mybir.AluOpType.subtract
MULT = mybir.AluOpType.mult
BYPASS = mybir.AluOpType.bypass
EXP = mybir.ActivationFunctionType.Exp
SQUARE = mybir.ActivationFunctionType.Square
SQRT = mybir.ActivationFunctionType.Sqrt
IDENT = mybir.ActivationFunctionType.Identity


def _patch_tile_drain():
    """This container's walrus accepts at most 1 sync-wait per instruction
    (2 for EventSemaphore), but TileContext's final drain can carry several.
    Split the excess waits onto single-wait SP nops emitted after the drain
    (all complete before the all-engine barrier, so semantics are kept)."""
    if getattr(tile.TileContext, "_drain_patched", False):
        return
    from concourse.tile import ScopedClock

    def _drain_and_barrier(self, tick_clock, wait_clock):
        nc = self.nc
        drain_inst = nc.sync.drain()
        wait_clock.add_sem_waits(
            drain_inst.ins, ScopedClock({None: tick_clock.global_clock})
        )
        si = drain_inst.ins.sync_info
        waits = list(si.on_wait or [])
        if len(waits) > 1:
            si.on_wait = waits[:1]
            for w in waits[1:]:
                nop = nc.sync.nop(nofuse=True)
                nsi = nop.ins.sync_info
                if nsi is None:
                    nop.ins.sync_info = mybir.SyncInfo(on_wait=[w], on_update=[])
                else:
                    nsi.on_wait = (nsi.on_wait or []) + [w]
        nc.all_engine_barrier()
        popped = nc._tile_sem_poison_stack.pop()
        assert popped is self._sem_poison
        nc.clear_and_free_semaphores(list(self.sems.allocated().values()))
        nc.all_engine_barrier()

    tile.TileContext._drain_and_barrier = _drain_and_barrier
    tile.TileContext._drain_patched = True


def _split_multiwaits(nc):
    """This walrus supports at most 1 sync-wait per instruction; move excess
    waits onto single-wait NoOps inserted just before (same engine)."""
    for fn in nc.m.functions:
        for blk in fn.blocks:
            insts = list(blk.instructions)
            new = []
            changed = False
            for inst in insts:
                si = inst.sync_info
                if si is not None and si.on_wait and len(si.on_wait) > 1:
                    waits = list(si.on_wait)
                    for j, wcond in enumerate(waits[:-1]):
                        nop = mybir.InstNoOp(
                            name=f"{inst.name}-w{j}", engine=inst.engine,
                            ins=[], outs=[],
                            sync_info=mybir.SyncInfo(on_wait=[wcond],
                                                     on_update=[]))
                        new.append(nop)
                    si.on_wait = waits[-1:]
                    changed = True
                new.append(inst)
            if changed:
                blk.instructions = new


def build_nc(split_waits=True, reps=1):
    _patch_tile_drain()
    nc = bass.Bass("TRN2", target_bir_lowering=False, debug=False)

    x1t = nc.dram_tensor("x1t", [D, N], F32R, kind="ExternalInput").ap()
    x2t = nc.dram_tensor("x2t", [D, N], F32R, kind="ExternalInput").ap()
    wk = nc.dram_tensor("wk", [D, HPC * HD], F32R, kind="ExternalInput").ap()
    wv = nc.dram_tensor("wv", [D, HPC * HD], F32R, kind="ExternalInput").ap()
    wqg = nc.dram_tensor("wqg", [D, HPC * HD], F32R, kind="ExternalInput").ap()
    wo = nc.dram_tensor("wo", [P, HPC, D], F32R, kind="ExternalInput").ap()
    vecs = nc.dram_tensor("vecs", [HPC * HD, 4], F32, kind="ExternalInput").ap()
    cst = nc.dram_tensor("cst", [P, 2], F32R, kind="ExternalInput").ap()
    bo = nc.dram_tensor("bo", [D], F32, kind="ExternalInput").ap()
    outp = nc.dram_tensor("outp", [D, N], F32, kind="ExternalOutput").ap()

    x1r = x1t.rearrange("(c p) n -> c p n", p=P)
    x2r = x2t.rearrange("(c p) n -> c p n", p=P)
    outr = outp.rearrange("(c p) n -> c p n", p=P)

    with tile.TileContext(nc) as tc:
      for _rep in range(reps):
        # -------------------------------------------------- persistent pools
        with tc.tile_pool(name="glob", bufs=1) as gp, \
             tc.tile_pool(name="dram", bufs=1, space="DRAM") as dp:
            # weights
            sb_wk = gp.tile([P, DC, HPC * HD], F32R, tag="wk")
            nc.sync.dma_start(sb_wk, wk.rearrange("(c p) q -> p c q", p=P))
            sb_wv = gp.tile([P, DC, HPC * HD], F32R, tag="wv")
            nc.sync.dma_start(sb_wv, wv.rearrange("(c p) q -> p c q", p=P))
            sb_wqg = gp.tile([P, DC, HPC * HD], F32R, tag="wqg")
            nc.sync.dma_start(sb_wqg, wqg.rearrange("(c p) q -> p c q", p=P))
            sb_wo = gp.tile([P, HPC, D], F32R, tag="wo")
            nc.sync.dma_start(sb_wo, wo)
            # small per-partition constants: col 0 ones, 1 zero, 2 eps,
            # 3+h bk, 5+h bv, 7+h negg, 9+h cq, 11+c bo
            sb_c = gp.tile([P, 20], F32, tag="consts")
            nc.vector.memset(sb_c, 0.0)
            nc.vector.memset(sb_c[:, 0:1], 1.0)
            nc.vector.memset(sb_c[:, 2:3], EPS)
            nc.gpsimd.dma_start(
                sb_c[0:HD, 3:11].rearrange("p (k h) -> p k h", h=HPC),
                vecs.rearrange("(h p) k -> p k h", p=HD))
            nc.gpsimd.dma_start(sb_c[:, 11:11 + DC],
                                bo.rearrange("(c p) -> p c", p=P))
            sb_cst = gp.tile([P, 2], F32R, tag="cst")
            nc.sync.dma_start(sb_cst, cst)
            ones = sb_cst[:, 0:1]
            zro = sb_cst[:, 1:2]
            eps_b = sb_c[:, 2:3]

            ident = gp.tile([P, P], F32, tag="ident")
            make_identity(nc, ident)

            # activations kept for attention
            sb_q = gp.tile([P, HPC, N], F32R, tag="qT")
            nc.vector.tensor_copy(
                sb_q[HD:P, :, :],
                zro[HD:P, :, None].to_broadcast([P - HD, HPC, N]))
            sb_k = gp.tile([P, HPC, N], F32R, tag="kT")
            nc.vector.tensor_copy(
                sb_k[HD:P, :, :],
                zro[HD:P, :, None].to_broadcast([P - HD, HPC, N]))
            sb_v = gp.tile([P, HPC, NKC, HD + 1], F32R, tag="vnat")
            nc.vector.tensor_copy(
                sb_v[:, :, :, HD:HD + 1],
                ones[:, :, None, None].to_broadcast([P, HPC, NKC, 1]))

            dstats = dp.tile([2, N], F32, tag="dstats")
            ddens = []
            for i in range(4):
                dden_t = dp.tile([1, NQB], F32, tag=f"dden{i}", name=f"dden{i}")
                ddens.append(dden_t)



            # ------------------------------------------------ phase 1 + LN
            with tc.tile_pool(name="fpool", bufs=1) as fp:
                f_tiles = []
                stats = fp.tile([P, 2 * N], F32, tag="stats")

                with tc.tile_pool(name="p1", bufs=2) as p1, \
                     tc.tile_pool(name="x2p", bufs=2) as x2p, \
                     tc.tile_pool(name="x1p", bufs=2) as x1p, \
                     tc.tile_pool(name="vbp", bufs=3) as vbp, \
                     tc.tile_pool(name="sqp", bufs=2) as sqp, \
                     tc.tile_pool(name="stg", bufs=2) as stg, \
                     tc.tile_pool(name="ppj", bufs=1, space="PSUM") as ppj, \
                     tc.tile_pool(name="pps", bufs=1, space="PSUM") as pps, \
                     tc.tile_pool(name="ppt", bufs=2, space="PSUM") as ppt, \
                     tc.tile_pool(name="ppq", bufs=1, space="PSUM") as ppq:

                    # ---- neighborhood sums (DVE) per feature chunk
                    for c in range(DC):
                        x = x2p.tile([P, N], F32R, tag="x2")
                        nc.sync.dma_start(x, x2r[c])
                        fc = fp.tile([P, N], F32R, tag=f"f{c}")
                        f_tiles.append(fc)
                        cc = p1.tile([P, N], F32R, tag="ctmp")
                        # column (c-direction) 3-sum with zero edges
                        ceng = nc.gpsimd if c < 5 else nc.vector
                        ceng.tensor_tensor(cc[:, 0:N - 1], x[:, 0:N - 1],
                                           x[:, 1:N], op=ADD)
                        ceng.tensor_copy(cc[:, N - 1:N], x[:, N - 1:N])
                        ceng.tensor_tensor(cc[:, 1:N], cc[:, 1:N],
                                           x[:, 0:N - 1], op=ADD)
                        c3 = cc.rearrange("p (r g) -> p r g", g=G)
                        x3 = x.rearrange("p (r g) -> p r g", g=G)
                        # undo the wrap-around terms at the row seams
                        nc.vector.tensor_tensor(c3[:, 1:G, 0:1], c3[:, 1:G, 0:1],
                                                x3[:, 0:G - 1, G - 1:G], op=SUB)
                        nc.vector.tensor_tensor(c3[:, 0:G - 1, G - 1:G],
                                                c3[:, 0:G - 1, G - 1:G],
                                                x3[:, 1:G, 0:1], op=SUB)
                        # row (r-direction) 3-sum with zero edges
                        nc.any.tensor_tensor(fc[:, 0:N - G], cc[:, 0:N - G],
                                             cc[:, G:N], op=ADD)
                        nc.vector.tensor_copy(fc[:, N - G:N], cc[:, N - G:N])
                        nc.any.tensor_tensor(fc[:, G:N], fc[:, G:N],
                                             cc[:, 0:N - G], op=ADD)
                        # border corrections: F += w2 * x[max(r-1,0), max(c-1,0)]
                        f3 = fc.rearrange("p (r g) -> p r g", g=G)
                        stt = nc.vector.scalar_tensor_tensor
                        # top row r=0, c=1..47 (+3)
                        stt(f3[:, 0, 1:G], x3[:, 0, 0:G - 1], 3.0,
                            f3[:, 0, 1:G], op0=MULT, op1=ADD)
                        # bottom row r=47, c=1..47 (+3)
                        stt(f3[:, G - 1, 1:G], x3[:, G - 2, 0:G - 1], 3.0,
                            f3[:, G - 1, 1:G], op0=MULT, op1=ADD)
                        # left col c=0, r=1..47 (+3)
                        stt(f3[:, 1:G, 0:1], x3[:, 0:G - 1, 0:1], 3.0,
                            f3[:, 1:G, 0:1], op0=MULT, op1=ADD)
                        # right col c=47, r=1..46 (+3)
                        stt(f3[:, 1:G - 1, G - 1:G], x3[:, 0:G - 2, G - 2:G - 1],
                            3.0, f3[:, 1:G - 1, G - 1:G], op0=MULT, op1=ADD)
                        # corners: (0,0) +5; (0,47) +2; (47,0) +2; (47,47) +2
                        stt(f3[:, 0, 0:1], x3[:, 0, 0:1], 5.0,
                            f3[:, 0, 0:1], op0=MULT, op1=ADD)
                        stt(f3[:, 0, G - 1:G], x3[:, 0, G - 2:G - 1], 2.0,
                            f3[:, 0, G - 1:G], op0=MULT, op1=ADD)
                        stt(f3[:, G - 1, 0:1], x3[:, G - 2, 0:1], 2.0,
                            f3[:, G - 1, 0:1], op0=MULT, op1=ADD)
                        stt(f3[:, G - 1, G - 1:G], x3[:, G - 2, G - 2:G - 1], 2.0,
                            f3[:, G - 1, G - 1:G], op0=MULT, op1=ADD)

                    # ---- k/v projections from streamed x1 blocks (PE)
                    for nb in range(NB):
                        ns = slice(nb * NQB, (nb + 1) * NQB)
                        xb = x1p.tile([P, DC, NQB], F32R, tag="x1b")
                        for c in range(DC):
                            nc.sync.dma_start(xb[:, c, :], x1r[c][:, ns])
                        for h in range(HPC):
                            hs = slice(h * HD, (h + 1) * HD)
                            psk = ppj.tile([HD, NQB], F32, tag="pk")
                            psv = ppj.tile([HD, NQB], F32, tag="pv")
                            for c in range(DC):
                                nc.tensor.matmul(psk, (sb_wk[:, c, hs]),
                                                 (xb[:, c, :]),
                                                 start=(c == 0), stop=(c == DC - 1))
                            for c in range(DC):
                                nc.tensor.matmul(psv, (sb_wv[:, c, hs]),
                                                 (xb[:, c, :]),
                                                 start=(c == 0), stop=(c == DC - 1))
                            nc.scalar.activation(sb_k[0:HD, h, ns], psk,
                                                 IDENT,
                                                 bias=sb_c[0:HD, 3 + h:4 + h])
                            vb = vbp.tile([P, NQB], F32, tag="vblk")
                            nc.vector.memset(vb[HD:P, :], 0.0)
                            nc.scalar.activation(vb[0:HD, :], psv,
                                                 IDENT,
                                                 bias=sb_c[0:HD, 5 + h:6 + h])
                            # transpose v into natural [nk, hd] layout
                            for t in range(NQB // P):
                                kc = nb * (NQB // P) + t
                                pst = ppt.tile([P, P], F32, tag="ptr")
                                nc.tensor.transpose(pst, vb[:, t * P:(t + 1) * P],
                                                    ident)
                                nc.scalar.copy(sb_v[:, h, kc, 0:HD], pst[:, 0:HD])

                    # ---- LN statistics (sum and sum-of-squares over d)
                    for nb in range(NB):
                        ns = slice(nb * NQB, (nb + 1) * NQB)
                        psx = pps.tile([1, NQB], F32, tag="psx")
                        psq = pps.tile([1, NQB], F32, tag="psq")
                        for c in range(DC):
                            sq = sqp.tile([P, NQB], F32R, tag="sq")
                            nc.any.tensor_mul(sq, f_tiles[c][:, ns],
                                              f_tiles[c][:, ns])
                            nc.tensor.matmul(psx, (ones), (f_tiles[c][:, ns]),
                                             start=(c == 0), stop=(c == DC - 1))
                            nc.tensor.matmul(psq, (ones), (sq),
                                             start=(c == 0), stop=(c == DC - 1))
                        s1 = stg.tile([1, NQB], F32, tag="stg")
                        nc.any.tensor_copy(s1, psx)
                        nc.sync.dma_start(dstats[0:1, ns], s1)
                        s2 = stg.tile([1, NQB], F32, tag="stg")
                        nc.any.tensor_copy(s2, psq)
                        nc.sync.dma_start(dstats[1:2, ns], s2)

                    # ---- broadcast stats to all partitions, compute
                    #      a = rstd, b = mu * rstd (in place in `stats`)
                    nc.gpsimd.dma_start(
                        stats, dstats.rearrange("a n -> (a n)")[None, :]
                        .to_broadcast([P, 2 * N]))
                    mu = stats[:, 0:N]      # holds sum(x) -> later mu*rstd
                    vr = stats[:, N:2 * N]  # holds sum(x^2) -> later rstd
                    musq = p1.tile([P, N], F32, tag="ctmp")
                    # musq = (SX/D)^2 ; var = SQ/D - musq
                    nc.vector.scalar_tensor_tensor(musq, mu, 1.0 / (D * D), mu,
                                                   op0=MULT, op1=MULT)
                    nc.vector.scalar_tensor_tensor(vr, vr, 1.0 / D, musq,
                                                   op0=MULT, op1=SUB)
                    nc.scalar.activation(vr, vr, SQRT, bias=eps_b)
                    nc.vector.reciprocal(vr, vr)          # a = rstd
                    nc.vector.scalar_tensor_tensor(mu, mu, 1.0 / D, vr,
                                                   op0=MULT, op1=MULT)  # b

                    # ---- q projection with LN folded in
                    for nb in range(NB):
                        ns = slice(nb * NQB, (nb + 1) * NQB)
                        for h in range(HPC):
                            hs = slice(h * HD, (h + 1) * HD)
                            psq2 = ppq.tile([HD, NQB], F32, tag="pq")
                            for c in range(DC):
                                nc.tensor.matmul(psq2, (sb_wqg[:, c, hs]),
                                                 (f_tiles[c][:, ns]),
                                                 start=(c == 0), stop=(c == DC - 1))
                            qsl = sb_q[0:HD, h, ns]
                            nc.any.tensor_tensor(qsl, psq2, vr[0:HD, ns],
                                                 op=MULT)
                            nc.vector.scalar_tensor_tensor(
                                qsl, mu[0:HD, ns], sb_c[0:HD, 7 + h:8 + h], qsl,
                                op0=MULT, op1=ADD)
                            nc.any.tensor_scalar(qsl, qsl,
                                                 sb_c[0:HD, 9 + h:10 + h], None,
                                                 op0=ADD)

            # ---------------------------------------------------- attention
            with tc.tile_pool(name="att", bufs=2) as ap_, \
                 tc.tile_pool(name="ot", bufs=1) as otp, \
                 tc.tile_pool(name="den", bufs=2) as dnp, \
                 tc.tile_pool(name="ost", bufs=3) as osp, \
                 tc.tile_pool(name="ppk", bufs=2, space="PSUM") as ppk, \
                 tc.tile_pool(name="ppa", bufs=2, space="PSUM") as ppa, \
                 tc.tile_pool(name="ppw", bufs=2, space="PSUM") as ppw:

                sb_o = otp.tile([P, HPC, N], F32R, tag="oT")
                nc.vector.tensor_copy(
                    sb_o[HD:P, :, :],
                    zro[HD:P, :, None].to_broadcast([P - HD, HPC, N]))

                for nb in range(NB):
                    ns = slice(nb * NQB, (nb + 1) * NQB)
                    for h in range(HPC):
                        att = ap_.tile([P, NKC, NQB], F32R, tag="attT")
                        for kc2 in range(NKC // 2):
                            ps = ppk.tile([P, 2, 512], F32, tag="ps")
                            for j in range(2):
                                kc = kc2 * 2 + j
                                nc.tensor.matmul(
                                    ps[:, j, 0:NQB],
                                    (sb_k[:, h, kc * P:(kc + 1) * P]),
                                    (sb_q[:, h, ns]), start=True, stop=True)
                            nc.scalar.activation(att[:, 2 * kc2:2 * kc2 + 2, :],
                                                 ps[:, :, 0:NQB], EXP)
                        po = ppa.tile([HD + 1, NQB], F32, tag="po")
                        for kc in range(NKC):
                            nc.tensor.matmul(po, (sb_v[:, h, kc, :]),
                                             (att[:, kc, :]),
                                             start=(kc == 0), stop=(kc == NKC - 1))
                        d1 = dnp.tile([1, NQB], F32, tag="d1")
                        nc.vector.tensor_copy(d1, po[HD:HD + 1, :])
                        dden = ddens[(nb * HPC + h) % 4]
                        nc.sync.dma_start(dden, d1)
                        dr = dnp.tile([HD, NQB], F32, tag="dr")
                        nc.gpsimd.dma_start(dr, dden.to_broadcast([HD, NQB]))
                        nc.vector.reciprocal(dr, dr)
                        nc.any.tensor_tensor(sb_o[0:HD, h, ns], po[0:HD, :],
                                              dr, op=MULT)
                    # output projection for this n-block
                    for dc in range(DC):
                        pw = ppw.tile([P, NQB], F32, tag="pw")
                        for h in range(HPC):
                            nc.tensor.matmul(pw,
                                             (sb_wo[:, h, dc * P:(dc + 1) * P]),
                                             (sb_o[:, h, ns]),
                                             start=(h == 0), stop=(h == HPC - 1))
                        so = osp.tile([P, NQB], F32, tag="so")
                        nc.vector.tensor_scalar(so, pw,
                                                sb_c[:, 11 + dc:12 + dc], None,
                                                op0=ADD)
                        nc.sync.dma_start(outr[dc][:, ns], so)
    if split_waits:
        _split_multiwaits(nc)
    return nc


def make_core_inputs(inputs):
    """Host-side shard prep: slice/transpose weights, fold LN + q-scale."""
    x1 = np.ascontiguousarray(np.asarray(inputs["x1"], np.float32))
    x2 = np.ascontiguousarray(np.asarray(inputs["x2"], np.float32))
    WqT = np.asarray(inputs["Wq"], np.float32).T
    WkT = np.asarray(inputs["Wk"], np.float32).T
    WvT = np.asarray(inputs["Wv"], np.float32).T
    WoT = np.asarray(inputs["Wo"], np.float32).T
    bq = np.asarray(inputs["bq"], np.float32)
    bk = np.asarray(inputs["bk"], np.float32)
    bv = np.asarray(inputs["bv"], np.float32)
    bo = np.asarray(inputs["bo"], np.float32)
    gamma = np.asarray(inputs["ln_gamma"], np.float32)
    beta = np.asarray(inputs["ln_beta"], np.float32)

    x1t = [np.ascontiguousarray(x1[b].T) for b in range(B)]
    x2t = [np.ascontiguousarray(x2[b].T) for b in range(B)]

    cst_arr = np.zeros((P, 2), np.float32)
    cst_arr[:, 0] = 1.0
    in_maps = []
    for core in range(8):
        b, hp = divmod(core, 4)
        sl = slice(HPC * HD * hp, HPC * HD * (hp + 1))
        wq_s = (WqT[:, sl] * QSCALE).astype(np.float32)
        wqg = (gamma[:, None] * wq_s).astype(np.float32)
        negg = (-wqg.sum(axis=0)).astype(np.float32)
        cq = (beta @ wq_s + bq[sl] * QSCALE).astype(np.float32)
        vecs = np.stack([bk[sl], bv[sl], negg, cq], axis=1)  # [192, 4]
        wo_pad = np.zeros((P, HPC, D), np.float32)
        wo_pad[0:HD] = WoT[sl, :].reshape(HPC, HD, D).transpose(1, 0, 2)
        in_maps.append({
            "x1t": x1t[b],
            "x2t": x2t[b],
            "wk": np.ascontiguousarray(WkT[:, sl]),
            "wv": np.ascontiguousarray(WvT[:, sl]),
            "wqg": np.ascontiguousarray(wqg),
            "wo": wo_pad,
            "vecs": np.ascontiguousarray(vecs),
            "bo": bo if hp == 0 else np.zeros_like(bo),
            "cst": cst_arr,
        })
    return in_maps


def kernel(**inputs):
    in_maps = make_core_inputs(inputs)
    nc = build_nc()
    res = run_bass_kernel_spmd(nc, in_maps, core_ids=list(range(8)))
    outs = [r["outp"] for r in res.results]
    out = np.empty((B, N, D), np.float32)
    for b in range(B):
        acc = outs[4 * b] + outs[4 * b + 1]
        acc += outs[4 * b + 2]
        acc += outs[4 * b + 3]
        out[b] = acc.T
    return out



# revision 4
# speedup vs baseline: 18.3999x; 18.3999x over previous
"""Trainium2 Bass kernel v2 for nn_NeighborhoodSearch (sparse_attention).

Sharding: 8 cores = (batch b in {0,1}) x (head-pair hp in {0..3}); each core
computes a full-[N, D] partial contribution of its 2 heads through its slice
of Wo; the host sums the 4 partials per batch (and transposes back).

Math notes (validated against the reference in fp64):
 - The neighborhood "attention" softmax is over a singleton axis -> weights
   are all 1, so fused = sum of the 9 padded neighbors of x2 on the 48x48
   grid = boxsum3x3_zeropad(x2) + w2(r,c) * x2[max(r-1,0), max(c-1,0)]
   with w2 in {0, 2, 3, 5} (edge-replication correction).
 - LayerNorm feeds only the q projection, so it is folded into it:
   q = rstd[n] * (F @ (gamma*WqT*s)) - (rstd*mu)[n] * sum_d(gamma*WqT*s)
       + (beta @ WqT*s + bq*s)
 - bk drops exactly (softmax shift invariance: q.bk is constant per row).
 - bv folds exactly into the output bias: bo_eff = bv @ WoT_slice + bo.
 - Softmax needs no max-subtraction (scores are ~N(0, 0.3), |s| < ~3);
   the denominator comes free from an appended ones-column on v.

v2 deltas vs v1:
 - bf16 end-to-end on SBUF (psum stays fp32): halves DMA and doubles DVE.
 - v is produced directly in natural [key, hd] layout (stationary = x1
   chunk, moving = Wv for both heads): no PE transposes.
 - No DRAM round-trips: LN-stats and 1/den are broadcast across partitions
   with tiny PE matmuls (ones / selector stationaries).
 - Sum_d(F) comes free from a ones-column appended to the q weights.
"""

import sys

sys.path.insert(0, "/opt/trn_rl_repo")

import ml_dtypes
import numpy as np

import concourse.bass as bass
import concourse.mybir as mybir
import concourse.tile as tile
from concourse.bass_utils import run_bass_kernel_spmd

# ---------------------------------------------------------------- constants
B = 2
N = 2304          # sequence length = 48*48
D = 768           # model dim
G = 48            # grid side
P = 128           # partitions
DC = D // P       # 6 feature chunks
HD = 96           # head dim
HPC = 2           # heads per core
QW = HPC * HD     # 192
NQB = 384         # n-block width (matmul moving free dim)
NB = N // NQB     # 6 n-blocks
NKC = N // P      # 18 key chunks
EPS = 1e-5
QSCALE = HD ** -0.5
BANK = 512        # psum bank width in fp32 elements

F32 = mybir.dt.float32
F32R = mybir.dt.float32r
BF16 = mybir.dt.bfloat16
BF = ml_dtypes.bfloat16

ADD = mybir.AluOpType.add
SUB = mybir.AluOpType.subtract
MULT = mybir.AluOpType.mult
EXP = mybir.ActivationFunctionType.Exp
SQUARE = mybir.ActivationFunctionType.Square
SQRT = mybir.ActivationFunctionType.Sqrt
IDENT = mybir.ActivationFunctionType.Identity


def _patch_tile_drain():
    """This container's walrus accepts at most 1 sync-wait per instruction
    (2 for EventSemaphore), but TileContext's final drain can carry several.
    Split the excess waits onto single-wait SP nops emitted after the drain
    (all complete before the all-engine barrier, so semantics are kept)."""
    if getattr(tile.TileContext, "_drain_patched", False):
        return
    from concourse.tile import ScopedClock

    def _drain_and_barrier(self, tick_clock, wait_clock):
        nc = self.nc
        drain_inst = nc.sync.drain()
        wait_clock.add_sem_waits(
            drain_inst.ins, ScopedClock({None: tick_clock.global_clock})
        )
        si = drain_inst.ins.sync_info
        waits = list(si.on_wait or [])
        if len(waits) > 1:
            si.on_wait = waits[:1]
            for w in waits[1:]:
                nop = nc.sync.nop(nofuse=True)
                nsi = nop.ins.sync_info
                if nsi is None:
                    nop.ins.sync_info = mybir.SyncInfo(on_wait=[w], on_update=[])
                else:
                    nsi.on_wait = (nsi.on_wait or []) + [w]
        nc.all_engine_barrier()
        popped = nc._tile_sem_poison_stack.pop()
        assert popped is self._sem_poison
        nc.clear_and_free_semaphores(list(self.sems.allocated().values()))
        nc.all_engine_barrier()

    tile.TileContext._drain_and_barrier = _drain_and_barrier
    tile.TileContext._drain_patched = True


def _split_multiwaits(nc):
    """This walrus supports at most 1 sync-wait per instruction; move excess
    waits onto single-wait NoOps inserted just before (same engine)."""
    for fn in nc.m.functions:
        for blk in fn.blocks:
            insts = list(blk.instructions)
            new = []
            changed = False
            for inst in insts:
                si = inst.sync_info
                if si is not None and si.on_wait and len(si.on_wait) > 1:
                    waits = list(si.on_wait)
                    for j, wcond in enumerate(waits[:-1]):
                        nop = mybir.InstNoOp(
                            name=f"{inst.name}-w{j}", engine=inst.engine,
                            ins=[], outs=[],
                            sync_info=mybir.SyncInfo(on_wait=[wcond],
                                                     on_update=[]))
                        new.append(nop)
                    si.on_wait = waits[-1:]
                    changed = True
                new.append(inst)
            if changed:
                blk.instructions = new


def build_nc(split_waits=True, reps=1):
    _patch_tile_drain()
    nc = bass.Bass("TRN2", target_bir_lowering=False, debug=False)

    x1t = nc.dram_tensor("x1t", [D, N], BF16, kind="ExternalInput").ap()
    x2t = nc.dram_tensor("x2t", [D, N], BF16, kind="ExternalInput").ap()
    wk = nc.dram_tensor("wk", [D, QW], BF16, kind="ExternalInput").ap()
    wv = nc.dram_tensor("wv", [D, QW], BF16, kind="ExternalInput").ap()
    wqgo = nc.dram_tensor("wqgo", [D, HPC * (HD + 1)], BF16,
                          kind="ExternalInput").ap()
    wo = nc.dram_tensor("wo", [P, HPC * D], BF16, kind="ExternalInput").ap()
    cst = nc.dram_tensor("cst", [P, 16], F32, kind="ExternalInput").ap()
    outp = nc.dram_tensor("outp", [D, N], F32, kind="ExternalOutput").ap()

    x1v = x1t.rearrange("(c p) n -> p c n", p=P)    # [128, 6, 2304]
    x2v = x2t.rearrange("(c p) n -> c p n", p=P)    # [6][128, 2304]
    outv = outp.rearrange("(c p) n -> p c n", p=P)  # [128, 6, 2304]

    with tile.TileContext(nc) as tc:
      for _rep in range(reps):
        with tc.tile_pool(name="glob", bufs=1) as gp:
            # ------------------------------------------------ weights + consts
            sb_wk = gp.tile([P, DC, QW], BF16, tag="wk")
            nc.sync.dma_start(sb_wk, wk.rearrange("(c p) q -> p c q", p=P))
            sb_wv = gp.tile([P, DC, QW], BF16, tag="wv")
            nc.sync.dma_start(sb_wv, wv.rearrange("(c p) q -> p c q", p=P))
            sb_wq = gp.tile([P, DC, HPC, HD + 1], BF16, tag="wq")
            nc.sync.dma_start(
                sb_wq, wqgo.rearrange("(c p) (h q) -> p c h q", p=P, h=HPC))
            sb_wo = gp.tile([P, HPC, D], BF16, tag="wo")
            nc.sync.dma_start(sb_wo, wo.rearrange("p (h d) -> p h d", h=HPC))
            scst = gp.tile([P, 16], F32, tag="cst")
            nc.sync.dma_start(scst, cst)
            # scst columns: 0 eps | 1,2 negg(h) | 3,4 cq(h) | 5..10 bo(dc)

            ones1 = gp.tile([1, P], BF16, tag="ones1")
            nc.vector.memset(ones1, 1.0)
            onesP = gp.tile([P, 1], BF16, tag="onesP")
            nc.vector.memset(onesP, 1.0)

            # ------------------------------------------------ activations
            sb_k = gp.tile([P, HPC, N], BF16, tag="kT")
            nc.gpsimd.memset(sb_k[HD:P, :, :], 0.0)
            sb_q = gp.tile([P, HPC, N], BF16, tag="qT")
            nc.gpsimd.memset(sb_q[HD:P, :, :], 0.0)
            sb_o = gp.tile([P, HPC, N], BF16, tag="oT")
            nc.gpsimd.memset(sb_o[HD:P, :, :], 0.0)
            sb_v = gp.tile([P, NKC, HPC, HD + 1], BF16, tag="vnat")
            nc.gpsimd.memset(sb_v[:, :, :, HD:HD + 1], 1.0)
            fT = gp.tile([P, DC, N], BF16, tag="f")

            # ============================================ phase A: f, k, v
            with tc.tile_pool(name="pa_x", bufs=1) as xp, \
                 tc.tile_pool(name="pa_t", bufs=2) as tp, \
                 tc.tile_pool(name="pa_tp", bufs=1) as tpp, \
                 tc.tile_pool(name="pa_pk", bufs=2, space="PSUM") as pkp, \
                 tc.tile_pool(name="pa_pv", bufs=2, space="PSUM") as pvp:
                x1s = xp.tile([P, DC, N], BF16, tag="x1")
                x2s = xp.tile([P, DC, N], BF16, tag="x2")
                # interleave so both consumers start early; x2 c=5 first
                # (Pool's chunk)
                x2_order = (5, 0, 1, 2, 3, 4)
                for i in range(NB):
                    nc.sync.dma_start(x2s[:, x2_order[i], :], x2v[x2_order[i]])
                    ns = slice(i * NQB, (i + 1) * NQB)
                    nc.sync.dma_start(x1s[:, :, ns], x1v[:, :, ns])

                # ---- neighborhood sums; Pool is ~2.6x slower per element,
                # so it gets one chunk and DVE five
                for c in (5, 0, 1, 2, 3, 4):
                    eng = nc.gpsimd if c == 5 else nc.vector
                    x = x2s[:, c, :]
                    fc = fT[:, c, :]
                    pool = tpp if c == 5 else tp
                    cc = pool.tile([P, N], BF16, tag="ctmp")
                    # column (c-direction) 3-sum with zero edges
                    eng.tensor_tensor(cc[:, 0:N - 1], x[:, 0:N - 1],
                                      x[:, 1:N], op=ADD)
                    eng.tensor_copy(cc[:, N - 1:N], x[:, N - 1:N])
                    eng.tensor_tensor(cc[:, 1:N], cc[:, 1:N],
                                      x[:, 0:N - 1], op=ADD)
                    c3 = cc.rearrange("p (r g) -> p r g", g=G)
                    x3 = x.rearrange("p (r g) -> p r g", g=G)
                    # undo the wrap-around terms at the row seams
                    eng.tensor_tensor(c3[:, 1:G, 0:1], c3[:, 1:G, 0:1],
                                      x3[:, 0:G - 1, G - 1:G], op=SUB)
                    eng.tensor_tensor(c3[:, 0:G - 1, G - 1:G],
                                      c3[:, 0:G - 1, G - 1:G],
                                      x3[:, 1:G, 0:1], op=SUB)
                    # row (r-direction) 3-sum with zero edges
                    eng.tensor_tensor(fc[:, 0:N - G], cc[:, 0:N - G],
                                      cc[:, G:N], op=ADD)
                    eng.tensor_copy(fc[:, N - G:N], cc[:, N - G:N])
                    eng.tensor_tensor(fc[:, G:N], fc[:, G:N],
                                      cc[:, 0:N - G], op=ADD)
                    # border corrections: F += w2 * x[max(r-1,0), max(c-1,0)]
                    # (TensorScalarPtr is not ISA-legal on Pool -> always DVE)
                    f3 = fc.rearrange("p (r g) -> p r g", g=G)
                    stt = nc.vector.scalar_tensor_tensor
                    stt(f3[:, 0, 1:G], x3[:, 0, 0:G - 1], 3.0,
                        f3[:, 0, 1:G], op0=MULT, op1=ADD)
                    stt(f3[:, G - 1, 1:G], x3[:, G - 2, 0:G - 1], 3.0,
                        f3[:, G - 1, 1:G], op0=MULT, op1=ADD)
                    stt(f3[:, 1:G, 0:1], x3[:, 0:G - 1, 0:1], 3.0,
                        f3[:, 1:G, 0:1], op0=MULT, op1=ADD)
                    stt(f3[:, 1:G - 1, G - 1:G], x3[:, 0:G - 2, G - 2:G - 1],
                        3.0, f3[:, 1:G - 1, G - 1:G], op0=MULT, op1=ADD)
                    stt(f3[:, 0, 0:1], x3[:, 0, 0:1], 5.0,
                        f3[:, 0, 0:1], op0=MULT, op1=ADD)
                    stt(f3[:, 0, G - 1:G], x3[:, 0, G - 2:G - 1], 2.0,
                        f3[:, 0, G - 1:G], op0=MULT, op1=ADD)
                    stt(f3[:, G - 1, 0:1], x3[:, G - 2, 0:1], 2.0,
                        f3[:, G - 1, 0:1], op0=MULT, op1=ADD)
                    stt(f3[:, G - 1, G - 1:G], x3[:, G - 2, G - 2:G - 1], 2.0,
                        f3[:, G - 1, G - 1:G], op0=MULT, op1=ADD)

                # ---- k projection (hd-major), psum copy-out on ACT
                for nb in range(NB):
                    ns = slice(nb * NQB, (nb + 1) * NQB)
                    for h in range(HPC):
                        pk = pkp.tile([HD, BANK], F32, tag="pk")
                        for c in range(DC):
                            nc.tensor.matmul(pk[:, 0:NQB],
                                             sb_wk[:, c, h * HD:(h + 1) * HD],
                                             x1s[:, c, ns],
                                             start=(c == 0), stop=(c == DC - 1))
                        nc.scalar.copy(sb_k[0:HD, h, ns], pk[:, 0:NQB])

                # ---- v projection directly in natural [key, hd] layout
                for kc in range(NKC):
                    ks = slice(kc * P, (kc + 1) * P)
                    pv = pvp.tile([P, BANK], F32, tag="pv")
                    for c in range(DC):
                        nc.tensor.matmul(pv[:, 0:QW], x1s[:, c, ks],
                                         sb_wv[:, c, :],
                                         start=(c == 0), stop=(c == DC - 1))
                    nc.scalar.copy(
                        sb_v[:, kc, :, 0:HD],
                        pv[:, 0:QW].rearrange("p (h q) -> p h q", h=HPC))

            # ============================================ phase A5: LN + q
            with tc.tile_pool(name="a5_s", bufs=2) as sp5, \
                 tc.tile_pool(name="a5_sq", bufs=3) as sqp, \
                 tc.tile_pool(name="a5_pq", bufs=2, space="PSUM") as pqp, \
                 tc.tile_pool(name="a5_pb", bufs=1, space="PSUM") as pbp, \
                 tc.tile_pool(name="a5_pm", bufs=2, space="PSUM") as pmp:

                def a5_qmm(nb):
                    ns = slice(nb * NQB, (nb + 1) * NQB)
                    # q-projection; stationary col 96 is ones -> row 96 = SX
                    pq = pqp.tile([HD + 1, HPC, BANK], F32, tag="pq")
                    for h in range(HPC):
                        for c in range(DC):
                            nc.tensor.matmul(pq[:, h, 0:NQB],
                                             sb_wq[:, c, h, :],
                                             fT[:, c, ns],
                                             start=(c == 0), stop=(c == DC - 1))
                    # sum of squares (DVE: bf16 sbuf tt runs 2x there)
                    sq = sqp.tile([P, DC, NQB], BF16, tag="sq")
                    for c in range(DC):
                        nc.vector.tensor_tensor(sq[:, c, :], fT[:, c, ns],
                                                fT[:, c, ns], op=MULT)
                    pss = pmp.tile([1, BANK], F32, tag="pss")
                    for c in range(DC):
                        nc.tensor.matmul(pss[:, 0:NQB], onesP, sq[:, c, :],
                                         start=(c == 0), stop=(c == DC - 1))
                    return pq, pss

                def a5_finish(nb, pq, pss):
                    ns = slice(nb * NQB, (nb + 1) * NQB)
                    # gather SX/SQ, broadcast to all partitions via PE
                    st_sx = sp5.tile([1, NQB], BF16, tag="stx")
                    nc.scalar.copy(st_sx, pq[HD:HD + 1, 0, 0:NQB])
                    st_sq = sp5.tile([1, NQB], BF16, tag="stq")
                    nc.scalar.copy(st_sq, pss[:, 0:NQB])
                    pst = pbp.tile([P, 2, BANK], F32, tag="pst")
                    nc.tensor.matmul(pst[:, 0, 0:NQB], ones1, st_sx,
                                     start=True, stop=True)
                    nc.tensor.matmul(pst[:, 1, 0:NQB], ones1, st_sq,
                                     start=True, stop=True)
                    # pull both psum reads to the chain head so pst frees
                    # early (bufs=1): mu = SX/D, sqd = SQ/D
                    mu_sb = sp5.tile([P, NQB], F32, tag="mu")
                    nc.vector.tensor_scalar(mu_sb, pst[:, 0, 0:NQB], 1.0 / D,
                                            None, op0=MULT)
                    sqd_sb = sp5.tile([P, NQB], F32, tag="sqd")
                    nc.vector.tensor_scalar(sqd_sb, pst[:, 1, 0:NQB], 1.0 / D,
                                            None, op0=MULT)
                    # rstd = 1/sqrt(sqd - mu^2 + eps); tb = mu * rstd
                    musq = sp5.tile([P, NQB], F32, tag="musq")
                    nc.scalar.activation(musq, mu_sb, SQUARE)
                    varb = sp5.tile([P, NQB], F32, tag="varb")
                    nc.vector.tensor_tensor(varb, sqd_sb, musq, op=SUB)
                    sd = sp5.tile([P, NQB], F32, tag="sd")
                    nc.scalar.activation(sd, varb, SQRT, bias=scst[:, 0:1])
                    rstd = sp5.tile([P, NQB], F32, tag="rstd")
                    nc.vector.reciprocal(rstd, sd)
                    tb = sp5.tile([P, NQB], F32, tag="tb")
                    nc.vector.tensor_tensor(tb, mu_sb, rstd, op=MULT)
                    # q = rstd*pq + (tb*negg + cq); ts/tt keep DVE fast modes
                    # (scalar_tensor_tensor never does)
                    for h in range(HPC):
                        tb2 = sp5.tile([HD, NQB], BF16, tag=f"tb2{h}")
                        nc.vector.tensor_scalar(tb2, tb[0:HD, :],
                                                scst[0:HD, 1 + h:2 + h], None,
                                                op0=MULT)
                        nc.vector.tensor_scalar(tb2, tb2,
                                                scst[0:HD, 3 + h:4 + h], None,
                                                op0=ADD)
                        qsl = sb_q[0:HD, h, ns]
                        nc.vector.tensor_tensor(qsl, pq[0:HD, h, 0:NQB],
                                                rstd[0:HD, :], op=MULT)
                        nc.gpsimd.tensor_tensor(qsl, qsl, tb2, op=ADD)

                live = a5_qmm(0)
                for nb in range(NB):
                    nxt = a5_qmm(nb + 1) if nb + 1 < NB else None
                    a5_finish(nb, *live)
                    live = nxt

            # ============================================ phase B: attention
            with tc.tile_pool(name="b_att", bufs=2) as atp, \
                 tc.tile_pool(name="b_sm", bufs=2) as smp, \
                 tc.tile_pool(name="b_out", bufs=2) as otp, \
                 tc.tile_pool(name="b_pk", bufs=2, space="PSUM") as ppk, \
                 tc.tile_pool(name="b_po", bufs=2, space="PSUM") as ppo, \
                 tc.tile_pool(name="b_px", bufs=2, space="PSUM") as ppx:
                for nb in range(NB):
                    ns = slice(nb * NQB, (nb + 1) * NQB)
                    atts = []
                    for h in range(HPC):
                        att = atp.tile([P, NKC, NQB], BF16, tag=f"att{h}")
                        atts.append(att)
                        for kc2 in range(NKC // 2):
                            ps = ppk.tile([P, 2, BANK], F32, tag="ps")
                            for j in range(2):
                                kc = kc2 * 2 + j
                                nc.tensor.matmul(
                                    ps[:, j, 0:NQB],
                                    sb_k[:, h, kc * P:(kc + 1) * P],
                                    sb_q[:, h, ns], start=True, stop=True)
                            nc.scalar.activation(
                                att[:, 2 * kc2:2 * kc2 + 2, :],
                                ps[:, :, 0:NQB], EXP)
                    pos = []
                    for h in range(HPC):
                        po = ppo.tile([HD + 1, BANK], F32, tag="po")
                        pos.append(po)
                        for kc in range(NKC):
                            nc.tensor.matmul(po[:, 0:NQB], sb_v[:, kc, h, :],
                                             atts[h][:, kc, :],
                                             start=(kc == 0),
                                             stop=(kc == NKC - 1))
                    for h in range(HPC):
                        po = pos[h]
                        dr1 = smp.tile([1, NQB], BF16, tag="dr1")
                        with nc.allow_low_precision(reason="f32r == f32 bits"):
                            nc.vector.reciprocal(dr1, po[HD:HD + 1, 0:NQB])
                        drb = ppx.tile([P, BANK], F32, tag="aux")
                        nc.tensor.matmul(drb[0:HD, 0:NQB], ones1[:, 0:HD], dr1,
                                         start=True, stop=True)
                        osl = sb_o[0:HD, h, ns]
                        nc.vector.tensor_copy(osl, po[0:HD, 0:NQB])
                        nc.vector.tensor_tensor(osl, osl, drb[0:HD, 0:NQB],
                                                op=MULT)
                    # output projection + bias, one DMA per n-block
                    ostg = otp.tile([P, DC, NQB], F32, tag="ostg")
                    for dc in range(DC):
                        pw = ppx.tile([P, BANK], F32, tag="aux")
                        for h in range(HPC):
                            nc.tensor.matmul(pw[:, 0:NQB],
                                             sb_wo[:, h, dc * P:(dc + 1) * P],
                                             sb_o[:, h, ns],
                                             start=(h == 0), stop=(h == HPC - 1))
                        nc.vector.tensor_scalar(ostg[:, dc, :], pw[:, 0:NQB],
                                                scst[:, 5 + dc:6 + dc], None,
                                                op0=ADD)
                    nc.sync.dma_start(outv[:, :, ns], ostg)
    if split_waits:
        _split_multiwaits(nc)
    return nc


def make_core_inputs(inputs):
    """Host-side shard prep: slice/transpose weights, fold LN + q-scale +
    biases; ship bf16."""
    x1 = np.asarray(inputs["x1"], np.float32)
    x2 = np.asarray(inputs["x2"], np.float32)
    WqT = np.asarray(inputs["Wq"], np.float32).T
    WkT = np.asarray(inputs["Wk"], np.float32).T
    WvT = np.asarray(inputs["Wv"], np.float32).T
    WoT = np.asarray(inputs["Wo"], np.float32).T
    bq = np.asarray(inputs["bq"], np.float32)
    bv = np.asarray(inputs["bv"], np.float32)
    bo = np.asarray(inputs["bo"], np.float32)
    gamma = np.asarray(inputs["ln_gamma"], np.float32)
    beta = np.asarray(inputs["ln_beta"], np.float32)
    # bk is exactly absorbed by softmax shift invariance -> dropped.

    x1t = [np.ascontiguousarray(x1[b].T.astype(BF)) for b in range(B)]
    x2t = [np.ascontiguousarray(x2[b].T.astype(BF)) for b in range(B)]

    in_maps = []
    for core in range(8):
        b, hp = divmod(core, 4)
        sl = slice(QW * hp, QW * (hp + 1))
        wq_s = (WqT[:, sl] * QSCALE).astype(np.float32)
        wqg = (gamma[:, None] * wq_s).astype(BF)          # rounded to bf16
        wqgo = np.zeros((D, HPC, HD + 1), np.float32)
        wqgo[:, :, 0:HD] = wqg.astype(np.float32).reshape(D, HPC, HD)
        wqgo[:, :, HD] = 1.0
        negg = -wqg.astype(np.float32).reshape(D, HPC, HD).sum(axis=0)  # [2,96]
        cq = (beta @ wq_s + bq[sl] * QSCALE).reshape(HPC, HD)
        wo_pad = np.zeros((P, HPC * D), np.float32)
        wo_pad[0:HD] = (WoT[sl, :].reshape(HPC, HD, D).transpose(1, 0, 2)
                        .reshape(HD, HPC * D))
        bo_eff = (bv[sl] @ WoT[sl, :]).astype(np.float32)
        if hp == 0:
            bo_eff = bo_eff + bo
        cst_arr = np.zeros((P, 16), np.float32)
        cst_arr[:, 0] = EPS
        cst_arr[0:HD, 1] = negg[0]
        cst_arr[0:HD, 2] = negg[1]
        cst_arr[0:HD, 3] = cq[0]
        cst_arr[0:HD, 4] = cq[1]
        cst_arr[:, 5:11] = bo_eff.reshape(DC, P).T
        in_maps.append({
            "x1t": x1t[b],
            "x2t": x2t[b],
            "wk": np.ascontiguousarray(WkT[:, sl]).astype(BF),
            "wv": np.ascontiguousarray(WvT[:, sl]).astype(BF),
            "wqgo": wqgo.reshape(D, HPC * (HD + 1)).astype(BF),
            "wo": wo_pad.astype(BF),
            "cst": cst_arr,
        })
    return in_maps


def kernel(**inputs):
    in_maps = make_core_inputs(inputs)
    nc = build_nc()
    res = run_bass_kernel_spmd(nc, in_maps, core_ids=list(range(8)))
    outs = [r["outp"] for r in res.results]
    out = np.empty((B, N, D), np.float32)
    for b in range(B):
        acc = outs[4 * b] + outs[4 * b + 1]
        acc += outs[4 * b + 2]
        acc += outs[4 * b + 3]
        out[b] = acc.T
    return out
